# revision 1
# baseline (speedup 1.0000x reference)
"""CIN (Compressed Interaction Network) forward kernel for 8 Trainium2 NeuronCores.

Reference computation (per batch b, embedding dim d):
    x0 = inputs[b, :, d]                 # [F=39]
    h0 = x0
    for k in 0..2:
        z  = outer(x0, h_{k})            # [F * Hk]
        h_{k+1} = z @ Wk + bk            # [256]
    out[b] = concat_k sum_d h_{k+1}      # [768]

Strategy: data-parallel over batch (64 per core).  Per core, rows r = (b, d)
are 2048 GEMM rows.  Everything is laid out transposed: x0T[f, r], hT[u, r].
The Khatri-Rao product z_T[(i,j), r] = x0T[i, r] * hT[j, r] is materialized
k-tile by k-tile on the Vector engine (fp16 -> 2x mode) from a DMA-broadcast
copy of x0T[i] and consumed immediately by the Tensor engine as the moving
operand of [K,512]-shaped matmuls accumulating into PSUM.  Weights (host
pre-cast to fp16, pre-tiled [128, KT, 256]) are the stationary operand.
The d-sum for the output is taken directly from PSUM (fp32) on the Vector
engine; the fp16 rounding of h only affects the recurrence, not the output
path.  Biases are all-zero in this model but are honored: device-side via
the ScalarE PSUM-evacuation (bias feeds the recurrence), host-side (exact)
for the D * b_k contribution to the pooled output.
"""

import os
import sys

import numpy as np

for _p in ("/opt/trn_rl_repo", "/root/.axon_site/_ro/trn_rl_repo"):
    if os.path.isdir(_p) and _p not in sys.path:
        sys.path.insert(0, _p)

N_CORES = 8
B, F, D = 512, 39, 32
U = 256
BL = B // N_CORES          # 64 batches per core
R = BL * D                 # 2048 GEMM rows per core
NB = 512                   # matmul moving free-dim (one PSUM bank of fp32)
NRB = R // NB              # 4 row blocks
K0 = F * F                 # 1521
KT0 = 13                   # layer-0 k-tiles: 3 i-values x 42 j-slots = 126 rows each
FP = 42                    # padded field count (x0 padded with 3 zero rows)
K12 = F * U                # 9984
KT12 = K12 // 128          # 78 k-tiles; kt = (i, half)

DT = "float16"             # device compute dtype for z / W / h ("float16" | "bfloat16")

_prog_cache = {}


def _np_dt():
    import ml_dtypes

    return np.float16 if DT == "float16" else ml_dtypes.bfloat16


def _build_program():
    import concourse.mybir as mybir
    from concourse import bacc, tile

    dt = mybir.dt
    cdt = getattr(dt, DT)
    f32 = dt.float32

    nc = bacc.Bacc(
        "TRN2", target_bir_lowering=False, debug=False, num_devices=N_CORES
    )
    x0_p = nc.declare_dram_parameter("x0", [FP, R], cdt, isOutput=False)
    # x0 rows each replicated 42x in DRAM: broadcast DMAs read distinct
    # addresses (HBM bank spread) instead of hammering one 4KB row.
    x0r_p = nc.declare_dram_parameter("x0r", [F * FP, R], cdt, isOutput=False)
    w0_p = nc.declare_dram_parameter("w0", [128, KT0, U], cdt, isOutput=False)
    w1_p = nc.declare_dram_parameter("w1", [128, KT12, U], cdt, isOutput=False)
    w2_p = nc.declare_dram_parameter("w2", [128, KT12, U], cdt, isOutput=False)
    bias_p = nc.declare_dram_parameter("bias", [128, 4], f32, isOutput=False)
    out_p = nc.declare_dram_parameter("out", [128, 6, BL], f32, isOutput=True)

    with tile.TileContext(nc) as tc:
        with (
            tc.tile_pool(name="const", bufs=1) as constp,
            tc.tile_pool(name="wpool", bufs=1) as wpool,
            tc.tile_pool(name="xb", bufs=5) as xbp,
            tc.tile_pool(name="zp", bufs=4) as zp,
            tc.tile_pool(name="hp", bufs=1) as hp,
            tc.tile_pool(name="psum", bufs=1, space="PSUM") as psp,
        ):
            # broadcast DMAs source from DRAM (re-reading one SBUF partition
            # 128x serializes on its port) and alternate trigger engines so
            # both dynamic HW queues run in parallel.
            bcast_n = [0]

            def bcast(dst, src_ap):
                eng = nc.sync if bcast_n[0] % 2 == 0 else nc.scalar
                bcast_n[0] += 1
                eng.dma_start(dst, src_ap)

            out_sb = constp.tile([128, 6, BL], f32, tag="out")
            h_tiles = {
                (l, c): hp.tile([128, R], cdt, tag=f"h{l}{c}", name=f"h{l}{c}")
                for l in range(2)
                for c in range(2)
            }

            # ---- prologue, hand-ordered so the critical path clears first:
            # xi[0] + xj0 head the two queues, then the first W0 k-tiles, then
            # the remaining layer-0 xi tiles interleaved with W0/W1 chunks.
            xi0_tiles = []

            def xi0_dma(kt):
                xi = xbp.tile([128, R], cdt, tag="xi", name="xi0", bufs=14)
                bcast(xi[:63, :], x0r_p[3 * kt * FP : 3 * kt * FP + 63, :])
                bcast(xi[63:126, :], x0r_p[3 * kt * FP + 63 : 3 * kt * FP + 126, :])
                xi0_tiles.append(xi)

            xj0 = constp.tile([126, R], cdt, tag="xj0")
            w0 = wpool.tile([128, KT0, U], cdt, tag="w0")
            w1 = wpool.tile([128, KT12, U], cdt, tag="w1")
            bias = constp.tile([128, 4], f32, tag="bias")

            # first-consumed tensors go in small pieces so their completion
            # semaphores fire early (DMA engines fair-share in-flight work)
            xi00 = xbp.tile([128, R], cdt, tag="xi", name="xi00", bufs=14)
            nc.sync.dma_start(xi00[:63, :], x0r_p[0:63, :])
            nc.scalar.dma_start(xj0[0:FP, :], x0_p[:, :])
            nc.sync.dma_start(xi00[63:126, :], x0r_p[63:126, :])
            nc.scalar.dma_start(xj0[FP : 2 * FP, :], x0_p[:, :])
            nc.scalar.dma_start(xj0[2 * FP : 126, :], x0_p[: 126 - 2 * FP, :])
            xi0_tiles.append(xi00)
            nc.sync.dma_start(w0[:, :2, :], w0_p[:, :2, :])
            nc.scalar.dma_start(bias[:, :], bias_p[:, :])
            xi0_dma(1)
            nc.sync.dma_start(w0[:, 2:7, :], w0_p[:, 2:7, :])
            xi0_dma(2)
            nc.scalar.dma_start(w0[:, 7:, :], w0_p[:, 7:, :])
            # only W1 chunks 0-1 load during layer 0; the rest stream in layer 1
            w1_chunks = list(range(0, KT12, 13))
            for kt in range(3, KT0):
                xi0_dma(kt)
                if kt - 3 < 1:
                    lo = w1_chunks[kt - 3]
                    (nc.sync if kt % 2 else nc.scalar).dma_start(
                        w1[:, lo : lo + 13, :], w1_p[:, lo : lo + 13, :]
                    )

            # ---- PE warm-up: the HAM clock gate needs ~3.4us of sustained
            # matmul activity to unthrottle 1.2 -> 2.4 GHz.  Startup is
            # DMA-bound anyway, so burn dummy matmuls on garbage SBUF data
            # into a PSUM bank; the first real accumulation starts with
            # start=True, which clears the bank.
            warm_ps = psp.tile([128, NB], f32, tag="ps_0_0", name="warm_ps")
            for _ in range(42):
                nc.tensor.matmul(
                    warm_ps[:, :],
                    h_tiles[(0, 0)][:, :128],
                    h_tiles[(0, 0)][:, :NB],
                    start=True,
                    stop=True,
                )

            def make_x(i, nm):
                t = xbp.tile([128, R], cdt, tag="xi", name=nm, bufs=14)
                bcast(
                    t[:, :],
                    x0r_p[i * FP : i * FP + 32, :]
                    .unsqueeze(1)
                    .to_broadcast((32, 4, R)),
                )
                return t

            l1_pre = {i: make_x(i, f"l1x{i}") for i in (0, 1)}
            l2_pre = {}

            def do_layer(l, w_t, z_fn, kt_n, kt_hook=None):
                ps = [
                    [
                        psp.tile([128, NB], f32, tag=f"ps_{c}_{r}", name=f"ps_{c}_{r}")
                        for r in range(NRB)
                    ]
                    for c in range(2)
                ]
                for kt in range(kt_n):
                    if kt_hook is not None:
                        kt_hook(kt)
                    klen, z_t = z_fn(kt)
                    for c in range(2):
                        lhsT = w_t[:klen, kt, c * 128 : (c + 1) * 128]
                        for r in range(NRB):
                            nc.tensor.matmul(
                                ps[c][r][:, :],
                                lhsT,
                                z_t[:klen, r * NB : (r + 1) * NB],
                                start=(kt == 0),
                                stop=(kt == kt_n - 1),
                            )
                # evacuations first: they gate the next layer's TTs and free the
                # PSUM banks.  The d-sum for layers 0/1 reads the fp16 h tiles
                # and is DEFERRED into the next layer's loop (kt hook) so it
                # stays off the boundary-critical DVE path.  Layer 2 has no h
                # tile, so its d-sum reads PSUM directly (no successor anyway).
                if l < 2:
                    for c in range(2):
                        for r in range(NRB):
                            # PSUM -> SBUF fp16 with per-partition bias; c=0 on
                            # DVE (same-engine gate for the next layer's first
                            # TTs), c=1 on the otherwise-idle Scalar engine so
                            # both halves evacuate in parallel at the boundary.
                            if c == 0:
                                nc.vector.tensor_scalar_add(
                                    h_tiles[(l, c)][:, r * NB : (r + 1) * NB],
                                    ps[c][r][:, :],
                                    bias[:, l * 2 + c : l * 2 + c + 1],
                                )
                            else:
                                nc.scalar.activation(
                                    h_tiles[(l, c)][:, r * NB : (r + 1) * NB],
                                    ps[c][r][:, :],
                                    mybir.ActivationFunctionType.Identity,
                                    bias=bias[:, l * 2 + c : l * 2 + c + 1],
                                )
                else:
                    for c in range(2):
                        for r in range(NRB):
                            nc.vector.tensor_reduce(
                                out_sb[:, l * 2 + c, r * (NB // D) : (r + 1) * (NB // D)],
                                ps[c][r].rearrange("p (b d) -> p b d", d=D),
                                axis=mybir.AxisListType.X,
                                op=mybir.AluOpType.add,
                            )

            def h_reduce(l):
                for c in range(2):
                    nc.vector.tensor_reduce(
                        out_sb[:, l * 2 + c, :],
                        h_tiles[(l, c)].rearrange("p (b d) -> p b d", d=D),
                        axis=mybir.AxisListType.X,
                        op=mybir.AluOpType.add,
                    )

            # ---- layer 0: k-tile t covers i in {3t, 3t+1, 3t+2} x 42 j-slots;
            # partition p = a*42 + jj; x0 rows 39..41 and the matching W0 rows
            # are zero padding, so the product is exactly 0 there. ----
            def z_layer0(kt):
                z_t = zp.tile([128, R], cdt, tag="z")
                nc.vector.tensor_mul(
                    z_t[:126, :], xi0_tiles[kt][:126, :], xj0[:126, :]
                )
                return 126, z_t

            do_layer(0, w0, z_layer0, KT0)

            # ---- layers 1, 2: z[(i, j), r] = x0[i, r] * h[j, r], k = i*256 + j ----
            def z_layer12(l, premade):
                xcur = [None]

                def fn(kt):
                    i, half = kt // 2, kt % 2
                    if half == 0:
                        if i in premade:
                            xcur[0] = premade[i]
                        else:
                            xcur[0] = make_x(i, "xi")
                    z_t = zp.tile([128, R], cdt, tag="z")
                    if kt < 2:
                        # boundary pipelining: slice-wise TT so each matmul's z
                        # slice is ready right after its h evacuation lands
                        for r in range(NRB):
                            nc.vector.tensor_mul(
                                z_t[:, r * NB : (r + 1) * NB],
                                xcur[0][:, r * NB : (r + 1) * NB],
                                h_tiles[(l - 1, half)][:, r * NB : (r + 1) * NB],
                            )
                    else:
                        nc.vector.tensor_mul(
                            z_t[:, :], xcur[0][:, :], h_tiles[(l - 1, half)][:, :]
                        )
                    return 128, z_t

                return fn

            w2 = wpool.tile([128, KT12, U], cdt, tag="w2")

            # stream the rest of W1 plus all of W2 at spread points in layer 1;
            # w1 chunk c is consumed starting at kt = 13c, w2 only in layer 2.
            w_sched = {0: (w1, w1_p, 1), 3: (w1, w1_p, 2), 8: (w1, w1_p, 3), 13: (w1, w1_p, 4),
                       20: (w1, w1_p, 5), 26: (w2, w2_p, 0), 34: (w2, w2_p, 1),
                       42: (w2, w2_p, 2), 50: (w2, w2_p, 3), 58: (w2, w2_p, 4),
                       64: (w2, w2_p, 5)}

            def w_hook(kt):
                if kt == 66:
                    l2_pre[0] = make_x(0, "l2x0")
                if kt == 70:
                    l2_pre[1] = make_x(1, "l2x1")
                if kt == 74:
                    l2_pre[2] = make_x(2, "l2x2")
                if kt == 76:
                    l2_pre[3] = make_x(3, "l2x3")
                if kt == 4:
                    h_reduce(0)   # deferred layer-0 d-sum, off the boundary path
                if kt == 6:
                    nc.sync.dma_start(out_p[:, 0:2, :], out_sb[:, 0:2, :])
                if kt in w_sched:
                    wt, wp, c = w_sched[kt]
                    lo = w1_chunks[c]
                    (nc.sync if c % 2 else nc.scalar).dma_start(
                        wt[:, lo : lo + 13, :], wp[:, lo : lo + 13, :]
                    )

            do_layer(1, w1, z_layer12(1, l1_pre), KT12, kt_hook=w_hook)

            def l2_hook(kt):
                if kt == 4:
                    h_reduce(1)   # deferred layer-1 d-sum
                if kt == 6:
                    nc.sync.dma_start(out_p[:, 2:4, :], out_sb[:, 2:4, :])

            do_layer(2, w2, z_layer12(2, l2_pre), KT12, kt_hook=l2_hook)

            nc.sync.dma_start(out_p[:, 4:6, :], out_sb[:, 4:6, :])

    nc.compile()
    return nc


def _get_program():
    if "nc" not in _prog_cache:
        _prog_cache["nc"] = _build_program()
    return _prog_cache["nc"]


def _prep_maps(inputs):
    cdt = _np_dt()
    x = np.asarray(inputs["inputs"], np.float32)          # [512, 39, 32]
    Ws = [np.asarray(inputs[f"W{k}"], np.float32) for k in range(3)]
    bs = [np.asarray(inputs[f"b{k}"], np.float32) for k in range(3)]

    # layer-0 weights: row (i, j) -> tile t = i//3, partition p = (i%3)*42 + j
    w0j = np.zeros((F, FP, U), np.float32)
    w0j[:, :F, :] = Ws[0].reshape(F, F, U)
    w0t = np.zeros((KT0, 128, U), np.float32)
    w0t[:, :126, :] = w0j.reshape(KT0, 3 * FP, U)
    w_tiled = [
        w0t.transpose(1, 0, 2).astype(cdt),
        Ws[1].reshape(KT12, 128, U).transpose(1, 0, 2).astype(cdt),
        Ws[2].reshape(KT12, 128, U).transpose(1, 0, 2).astype(cdt),
    ]
    w_tiled = [np.ascontiguousarray(w) for w in w_tiled]
    bias = np.zeros((128, 4), np.float32)
    for l in range(2):
        for c in range(2):
            bias[:, l * 2 + c] = bs[l][c * 128 : (c + 1) * 128]

    in_maps = []
    for core in range(N_CORES):
        xs = x[core * BL : (core + 1) * BL]               # [64, 39, 32]
        x0T = np.zeros((FP, R), cdt)
        x0T[:F] = xs.transpose(1, 0, 2).reshape(F, R).astype(cdt)
        x0r = np.ascontiguousarray(np.repeat(x0T[:F], FP, axis=0))
        in_maps.append(
            {
                "x0": x0T,
                "x0r": x0r,
                "w0": w_tiled[0],
                "w1": w_tiled[1],
                "w2": w_tiled[2],
                "bias": bias,
            }
        )
    return in_maps, bs


def _finish_output(results, bs):
    outs = []
    for core in range(N_CORES):
        o = np.asarray(results[core]["out"], np.float32)  # [128, 6, 64]
        outs.append(o.transpose(2, 1, 0).reshape(BL, 768))
    out = np.concatenate(outs, axis=0)
    for l in range(3):
        out[:, l * U : (l + 1) * U] += D * bs[l]
    return np.ascontiguousarray(out.astype(np.float32))


def kernel(**inputs) -> np.ndarray:
    from concourse.bass_utils import run_bass_kernel_spmd

    in_maps, bs = _prep_maps(inputs)
    nc = _get_program()
    res = run_bass_kernel_spmd(nc, in_maps, list(range(N_CORES))).results
    return _finish_output(res, bs)



# revision 2
# speedup vs baseline: 1.7974x; 1.7974x over previous
"""CIN (Compressed Interaction Network) forward kernel for 8 Trainium2 NeuronCores.

Reference computation (per batch b, embedding dim d):
    x0 = inputs[b, :, d]                 # [F=39]
    h0 = x0
    for k in 0..2:
        z  = outer(x0, h_{k})            # [F * Hk]
        h_{k+1} = z @ Wk + bk            # [256]
    out[b] = concat_k sum_d h_{k+1}      # [768]

Strategy: data-parallel over batch (64 per core).  Per core, rows r = (b, d)
are 2048 GEMM rows.  Layers 0 and 1 run transposed (x0T[f, r], hT[u, r]):
the Khatri-Rao product z_T[(i,j), r] = x0T[i, r] * hT[j, r] is materialized
k-tile by k-tile on the Vector engine (fp16 -> 2x mode) and consumed by the
Tensor engine as the moving operand of [K,512]-shaped matmuls into PSUM.

Layer 2 is NOT computed per-(b,d).  Its pooled output only needs
    out2[b,u] = sum_{i,j} W2[(i,j),u] * M2[b,i,j],
    M2[b,i,j] = sum_d x[b,i,d] * h2[b,j,d],
so instead of the 10.5 GFLOP full layer-2 GEMM we do:
  1. h2 [256, 2048] is transposed to h2t [r, u] by XBAR DMA transposes
     (no Tensor-engine time).
  2. M2 per 4-batch group g (128 rows): stationary = h2t chunk [128 r, 128 j],
     moving = block-diagonal X-tile [128 r, 160 (i,s)] (host-prepped; includes
     a ones-row i=39 whose columns yield out1 = sum_d h2 for free).
  3. out2 = 78-k-tile GEMM: stationary = M2 slices [128 j, 64 b], moving =
     W2 tiles [128, 256], accumulating into one [64, 256] PSUM region.
This replaces ~165us of layer-2 matmul with ~20us.

The d-sum for layer 0 is a Vector-engine reduce of the fp16 h1 tiles;
out1 comes from the M2 ones-row (fp32, copied straight from PSUM).
Biases: b0/b1 are added on-device during PSUM evacuation (they feed the
recurrence / M2); the D*b2 contribution to out2 is added host-side (exact).
"""

import os
import sys

import numpy as np

for _p in ("/opt/trn_rl_repo", "/root/.axon_site/_ro/trn_rl_repo"):
    if os.path.isdir(_p) and _p not in sys.path:
        sys.path.insert(0, _p)

N_CORES = 8
B, F, D = 512, 39, 32
U = 256
BL = B // N_CORES          # 64 batches per core
R = BL * D                 # 2048 GEMM rows per core
NB = 512                   # matmul moving free-dim (one PSUM bank of fp32)
NRB = R // NB              # 4 row blocks
K0 = F * F                 # 1521
KT0 = 13                   # layer-0 k-tiles: 3 i-values x 42 j-slots = 126 rows each
FP = 42                    # padded field count (x0 padded with 3 zero rows)
K12 = F * U                # 9984
KT12 = K12 // 128          # 78 k-tiles; kt = (i, half)
G = 16                     # 4-batch groups (128 rows each)
IW = F + 1                 # i-values in the M2 X-tile (39 + ones row)
N_DUMMY = 14               # PE keep-warm matmuls across the L1 -> M2 gap

DT = "float16"             # device compute dtype for z / W / h ("float16" | "bfloat16")

_prog_cache = {}


def _np_dt():
    import ml_dtypes

    return np.float16 if DT == "float16" else ml_dtypes.bfloat16


def _build_program():
    import concourse.mybir as mybir
    from concourse import bacc, tile

    dt = mybir.dt
    cdt = getattr(dt, DT)
    f32 = dt.float32

    nc = bacc.Bacc(
        "TRN2", target_bir_lowering=False, debug=False, num_devices=N_CORES
    )
    x0_p = nc.declare_dram_parameter("x0", [FP, R], cdt, isOutput=False)
    # x0 rows each replicated 42x in DRAM: broadcast DMAs read distinct
    # addresses (HBM bank spread) instead of hammering one 4KB row.
    x0r_p = nc.declare_dram_parameter("x0r", [F * FP, R], cdt, isOutput=False)
    w0_p = nc.declare_dram_parameter("w0", [128, KT0, U], cdt, isOutput=False)
    w1_p = nc.declare_dram_parameter("w1", [128, KT12, U], cdt, isOutput=False)
    w2_p = nc.declare_dram_parameter("w2", [128, KT12, U], cdt, isOutput=False)
    bias_p = nc.declare_dram_parameter("bias", [128, 4], f32, isOutput=False)
    xblk_p = nc.declare_dram_parameter("xblk", [128, G, 4 * IW], cdt, isOutput=False)
    out01_p = nc.declare_dram_parameter("out01", [128, 4, BL], f32, isOutput=True)
    out2_p = nc.declare_dram_parameter("out2", [BL, U], f32, isOutput=True)

    with tile.TileContext(nc) as tc:
        with (
            tc.tile_pool(name="const", bufs=1) as constp,
            tc.tile_pool(name="wpool", bufs=1) as wpool,
            tc.tile_pool(name="xb", bufs=5) as xbp,
            tc.tile_pool(name="zp", bufs=4) as zp,
            tc.tile_pool(name="hp", bufs=1) as hp,
            tc.tile_pool(name="psum", bufs=1, space="PSUM") as psp,
        ):
            # broadcast DMAs source from DRAM (re-reading one SBUF partition
            # 128x serializes on its port) and alternate trigger engines so
            # both dynamic HW queues run in parallel.
            bcast_n = [0]

            def bcast(dst, src_ap):
                eng = nc.sync if bcast_n[0] % 2 == 0 else nc.scalar
                bcast_n[0] += 1
                eng.dma_start(dst, src_ap)

            out_sb = constp.tile([128, 4, BL], f32, tag="out")
            h_tiles = {
                (l, c): hp.tile([128, R], cdt, tag=f"h{l}{c}", name=f"h{l}{c}")
                for l in range(2)
                for c in range(2)
            }
            h2t = {
                c: constp.tile([128, G, 128], cdt, tag=f"h2t{c}", name=f"h2t{c}")
                for c in range(2)
            }
            m2t = {
                jh: constp.tile([128, IW, BL], cdt, tag=f"m2t{jh}", name=f"m2t{jh}")
                for jh in range(2)
            }
            out2_sb = constp.tile([BL, U], f32, tag="out2sb")
            xblk = constp.tile([128, G, 4 * IW], cdt, tag="xblk")

            # all 8 PSUM banks, shared by every phase
            ps_all = {
                (c, r): psp.tile(
                    [128, NB], f32, tag=f"ps_{c}_{r}", name=f"ps_{c}_{r}"
                )
                for c in range(2)
                for r in range(NRB)
            }
            pslist = [ps_all[(c, r)] for c in range(2) for r in range(NRB)]

            # ---- prologue, hand-ordered so the critical path clears first:
            # xi[0] + xj0 head the two queues, then the first W0 k-tiles, then
            # the remaining layer-0 xi tiles interleaved with W0/W1 chunks.
            xi0_tiles = []

            def xi0_dma(kt):
                xi = xbp.tile([128, R], cdt, tag="xi", name="xi0", bufs=14)
                bcast(xi[:63, :], x0r_p[3 * kt * FP : 3 * kt * FP + 63, :])
                bcast(xi[63:126, :], x0r_p[3 * kt * FP + 63 : 3 * kt * FP + 126, :])
                xi0_tiles.append(xi)

            xj0 = constp.tile([126, R], cdt, tag="xj0")
            w0 = wpool.tile([128, KT0, U], cdt, tag="w0")
            w1 = wpool.tile([128, KT12, U], cdt, tag="w1")
            bias = constp.tile([128, 4], f32, tag="bias")

            # first-consumed tensors go in small pieces so their completion
            # semaphores fire early (DMA engines fair-share in-flight work)
            xi00 = xbp.tile([128, R], cdt, tag="xi", name="xi00", bufs=14)
            nc.sync.dma_start(xi00[:63, :], x0r_p[0:63, :])
            nc.scalar.dma_start(xj0[0:FP, :], x0_p[:, :])
            nc.sync.dma_start(xi00[63:126, :], x0r_p[63:126, :])
            nc.scalar.dma_start(xj0[FP : 2 * FP, :], x0_p[:, :])
            nc.scalar.dma_start(xj0[2 * FP : 126, :], x0_p[: 126 - 2 * FP, :])
            xi0_tiles.append(xi00)
            nc.sync.dma_start(w0[:, :2, :], w0_p[:, :2, :])
            nc.scalar.dma_start(bias[:, :], bias_p[:, :])
            xi0_dma(1)
            nc.sync.dma_start(w0[:, 2:7, :], w0_p[:, 2:7, :])
            xi0_dma(2)
            nc.scalar.dma_start(w0[:, 7:, :], w0_p[:, 7:, :])
            # only W1 chunks 0-1 load during layer 0; the rest stream in layer 1
            w1_chunks = list(range(0, KT12, 13))
            for kt in range(3, KT0):
                xi0_dma(kt)
                if kt - 3 < 1:
                    lo = w1_chunks[kt - 3]
                    (nc.sync if kt % 2 else nc.scalar).dma_start(
                        w1[:, lo : lo + 13, :], w1_p[:, lo : lo + 13, :]
                    )

            # ---- PE warm-up: the HAM clock gate needs ~3.4us of sustained
            # matmul activity to unthrottle 1.2 -> 2.4 GHz.  Startup is
            # DMA-bound anyway, so burn dummy matmuls on garbage SBUF data
            # into a PSUM bank; the first real accumulation starts with
            # start=True, which clears the bank.
            warm_ps = ps_all[(0, 0)]
            for _ in range(42):
                nc.tensor.matmul(
                    warm_ps[:, :],
                    h_tiles[(0, 0)][:, :128],
                    h_tiles[(0, 0)][:, :NB],
                    start=True,
                    stop=True,
                )

            def make_x(i, nm):
                t = xbp.tile([128, R], cdt, tag="xi", name=nm, bufs=14)
                bcast(
                    t[:, :],
                    x0r_p[i * FP : i * FP + 32, :]
                    .unsqueeze(1)
                    .to_broadcast((32, 4, R)),
                )
                return t

            l1_pre = {i: make_x(i, f"l1x{i}") for i in (0, 1)}

            def do_layer(l, w_t, z_fn, kt_n, kt_hook=None):
                ps = [[ps_all[(c, r)] for r in range(NRB)] for c in range(2)]
                for kt in range(kt_n):
                    if kt_hook is not None:
                        kt_hook(kt)
                    klen, z_t = z_fn(kt)
                    for c in range(2):
                        lhsT = w_t[:klen, kt, c * 128 : (c + 1) * 128]
                        for r in range(NRB):
                            nc.tensor.matmul(
                                ps[c][r][:, :],
                                lhsT,
                                z_t[:klen, r * NB : (r + 1) * NB],
                                start=(kt == 0),
                                stop=(kt == kt_n - 1),
                            )
                # evacuations: they gate the next phase and free the PSUM
                # banks.  PSUM -> SBUF fp16 with per-partition bias; c=0 on
                # DVE (same-engine gate for the next layer's first TTs), c=1
                # on the otherwise-idle Scalar engine so both halves evacuate
                # in parallel at the boundary.  Layer 1 goes r-descending:
                # the XBAR transposes + M2 matmuls consume high row blocks
                # first (g descending).
                rord = range(NRB) if l == 0 else range(NRB - 1, -1, -1)
                for c in range(2):
                    for r in rord:
                        if c == 0:
                            nc.vector.tensor_scalar_add(
                                h_tiles[(l, c)][:, r * NB : (r + 1) * NB],
                                ps[c][r][:, :],
                                bias[:, l * 2 + c : l * 2 + c + 1],
                            )
                        else:
                            nc.scalar.activation(
                                h_tiles[(l, c)][:, r * NB : (r + 1) * NB],
                                ps[c][r][:, :],
                                mybir.ActivationFunctionType.Identity,
                                bias=bias[:, l * 2 + c : l * 2 + c + 1],
                            )

            def h_reduce(l):
                for c in range(2):
                    nc.vector.tensor_reduce(
                        out_sb[:, l * 2 + c, :],
                        h_tiles[(l, c)].rearrange("p (b d) -> p b d", d=D),
                        axis=mybir.AxisListType.X,
                        op=mybir.AluOpType.add,
                    )

            # ---- layer 0: k-tile t covers i in {3t, 3t+1, 3t+2} x 42 j-slots;
            # partition p = a*42 + jj; x0 rows 39..41 and the matching W0 rows
            # are zero padding, so the product is exactly 0 there. ----
            def z_layer0(kt):
                z_t = zp.tile([128, R], cdt, tag="z")
                nc.vector.tensor_mul(
                    z_t[:126, :], xi0_tiles[kt][:126, :], xj0[:126, :]
                )
                return 126, z_t

            do_layer(0, w0, z_layer0, KT0)

            # ---- layer 1: z[(i, j), r] = x0[i, r] * h1[j, r], k = i*256 + j ----
            def z_layer1(premade):
                xcur = [None]

                def fn(kt):
                    i, half = kt // 2, kt % 2
                    if half == 0:
                        if i in premade:
                            xcur[0] = premade[i]
                        else:
                            xcur[0] = make_x(i, "xi")
                    z_t = zp.tile([128, R], cdt, tag="z")
                    if kt < 2:
                        # boundary pipelining: slice-wise TT so each matmul's z
                        # slice is ready right after its h evacuation lands
                        for r in range(NRB):
                            nc.vector.tensor_mul(
                                z_t[:, r * NB : (r + 1) * NB],
                                xcur[0][:, r * NB : (r + 1) * NB],
                                h_tiles[(0, half)][:, r * NB : (r + 1) * NB],
                            )
                    else:
                        nc.vector.tensor_mul(
                            z_t[:, :], xcur[0][:, :], h_tiles[(0, half)][:, :]
                        )
                    return 128, z_t

                return fn

            w2 = wpool.tile([128, KT12, U], cdt, tag="w2")

            # stream the rest of W1 plus all of W2 at spread points in layer 1;
            # w1 chunk c is consumed starting at kt = 13c, w2 only at the end.
            w_sched = {0: (w1, w1_p, 1), 3: (w1, w1_p, 2), 8: (w1, w1_p, 3), 13: (w1, w1_p, 4),
                       20: (w1, w1_p, 5), 26: (w2, w2_p, 0), 34: (w2, w2_p, 1),
                       42: (w2, w2_p, 2), 50: (w2, w2_p, 3), 58: (w2, w2_p, 4),
                       64: (w2, w2_p, 5)}

            def w_hook(kt):
                if kt == 4:
                    h_reduce(0)   # deferred layer-0 d-sum, off the boundary path
                if kt == 6:
                    nc.sync.dma_start(out01_p[:, 0:2, :], out_sb[:, 0:2, :])
                if kt == 68:
                    nc.scalar.dma_start(xblk[:, :, :], xblk_p[:, :, :])
                if kt in w_sched:
                    wt, wp, c = w_sched[kt]
                    lo = w1_chunks[c]
                    (nc.sync if c % 2 else nc.scalar).dma_start(
                        wt[:, lo : lo + 13, :], wp[:, lo : lo + 13, :]
                    )

            do_layer(1, w1, z_layer1(l1_pre), KT12, kt_hook=w_hook)

            # ---- layer-2 output path ----
            # 1) h2 [j, r] -> h2t [r, g, j] via XBAR DMA transpose, one call
            # per (c half, row block), following the evacuation order.
            tn = 0
            for r in range(NRB - 1, -1, -1):
                for c in range(2):
                    eng = nc.sync if tn % 2 == 0 else nc.scalar
                    tn += 1
                    eng.dma_start_transpose(
                        h2t[c][:, 4 * r : 4 * r + 4, :],
                        h_tiles[(1, c)][:, r * NB : (r + 1) * NB],
                    )

            # 2) keep the PE warm across the evac/transpose gap (HAM MID
            # window is ~3.4us); garbage matmuls into an unused PSUM region.
            for _ in range(N_DUMMY):
                nc.tensor.matmul(
                    pslist[7][:, 0:256],
                    h_tiles[(0, 0)][:, :128],
                    h_tiles[(0, 0)][:, :256],
                    start=True,
                    stop=True,
                )

            # 3) M2[b,i,j] = sum_d x[b,i,d] h2[b,j,d] per 4-batch group:
            # stationary = h2t chunk [128 r, 128 j], moving = block-diag
            # X-tile [128 r, 160 (i,s)].  Column i=39 is the ones-row -> out1.
            m2n = 0
            for g in range(G - 1, -1, -1):
                for jh in range(2):
                    pm = pslist[(2 * g + jh) % 7]
                    nc.tensor.matmul(
                        pm[:, 0:160],
                        h2t[jh][:, g, :],
                        xblk[:, g, :],
                        start=True,
                        stop=True,
                    )
                    if m2n % 2 == 0:
                        nc.vector.tensor_copy(
                            m2t[jh][:, :, 4 * g : 4 * g + 4],
                            pm[:, 0:160].rearrange("p (i s) -> p i s", s=4),
                        )
                        nc.vector.tensor_copy(
                            out_sb[:, 2 + jh, 4 * g : 4 * g + 4],
                            pm[:, 156:160],
                        )
                    else:
                        nc.scalar.activation(
                            m2t[jh][:, :, 4 * g : 4 * g + 4],
                            pm[:, 0:160].rearrange("p (i s) -> p i s", s=4),
                            mybir.ActivationFunctionType.Identity,
                        )
                        nc.scalar.activation(
                            out_sb[:, 2 + jh, 4 * g : 4 * g + 4],
                            pm[:, 156:160],
                            mybir.ActivationFunctionType.Identity,
                        )
                    m2n += 1

            # 4) out2[b,u] = sum over 78 k-tiles: stationary = M2 slice
            # [128 j, 64 b], moving = W2 tile [128, 256].
            for kt in range(KT12):
                i, jh = divmod(kt, 2)
                nc.tensor.matmul(
                    pslist[7][0:BL, 256:512],
                    m2t[jh][:, i, :],
                    w2[:, kt, :],
                    start=(kt == 0),
                    stop=(kt == KT12 - 1),
                )

            nc.vector.tensor_copy(out2_sb[:, :], pslist[7][0:BL, 256:512])
            nc.sync.dma_start(out01_p[:, 2:4, :], out_sb[:, 2:4, :])
            nc.scalar.dma_start(out2_p[:, :], out2_sb[:, :])

    nc.compile()
    return nc


def _get_program():
    if "nc" not in _prog_cache:
        _prog_cache["nc"] = _build_program()
    return _prog_cache["nc"]


def _prep_maps(inputs):
    cdt = _np_dt()
    x = np.asarray(inputs["inputs"], np.float32)          # [512, 39, 32]
    Ws = [np.asarray(inputs[f"W{k}"], np.float32) for k in range(3)]
    bs = [np.asarray(inputs[f"b{k}"], np.float32) for k in range(3)]

    # layer-0 weights: row (i, j) -> tile t = i//3, partition p = (i%3)*42 + j
    w0j = np.zeros((F, FP, U), np.float32)
    w0j[:, :F, :] = Ws[0].reshape(F, F, U)
    w0t = np.zeros((KT0, 128, U), np.float32)
    w0t[:, :126, :] = w0j.reshape(KT0, 3 * FP, U)
    w_tiled = [
        w0t.transpose(1, 0, 2).astype(cdt),
        Ws[1].reshape(KT12, 128, U).transpose(1, 0, 2).astype(cdt),
        Ws[2].reshape(KT12, 128, U).transpose(1, 0, 2).astype(cdt),
    ]
    w_tiled = [np.ascontiguousarray(w) for w in w_tiled]
    bias = np.zeros((128, 4), np.float32)
    for l in range(2):
        for c in range(2):
            bias[:, l * 2 + c] = bs[l][c * 128 : (c + 1) * 128]

    in_maps = []
    for core in range(N_CORES):
        xs = x[core * BL : (core + 1) * BL]               # [64, 39, 32]
        x0T = np.zeros((FP, R), cdt)
        x0T[:F] = xs.transpose(1, 0, 2).reshape(F, R).astype(cdt)
        x0r = np.ascontiguousarray(np.repeat(x0T[:F], FP, axis=0))
        # block-diagonal X-tile for M2: xblk[s*32+d, g, i*4+s] = x[4g+s, i, d]
        # (i=39 -> 1.0); other s-slots zero so each group's contraction stays
        # within its own batch.
        xg = xs.reshape(G, 4, F, D)                       # [g, s, i, d]
        xblk = np.zeros((128, G, 4 * IW), np.float32)
        for s in range(4):
            blk = np.zeros((D, G, IW), np.float32)
            blk[:, :, :F] = xg[:, s].transpose(2, 0, 1)   # [d, g, i]
            blk[:, :, F] = 1.0
            xblk[s * D : (s + 1) * D, :, s :: 4] = blk
        in_maps.append(
            {
                "x0": x0T,
                "x0r": x0r,
                "w0": w_tiled[0],
                "w1": w_tiled[1],
                "w2": w_tiled[2],
                "bias": bias,
                "xblk": np.ascontiguousarray(xblk.astype(cdt)),
            }
        )
    return in_maps, bs


def _finish_output(results, bs):
    outs = []
    for core in range(N_CORES):
        o = np.asarray(results[core]["out01"], np.float32)   # [128, 4, 64]
        o01 = o.transpose(2, 1, 0).reshape(BL, 2, U)         # [b, l, u]
        o2 = np.asarray(results[core]["out2"], np.float32)   # [64, 256]
        outs.append(
            np.concatenate([o01[:, 0, :], o01[:, 1, :], o2 + D * bs[2]], axis=1)
        )
    out = np.concatenate(outs, axis=0)
    return np.ascontiguousarray(out.astype(np.float32))


def kernel(**inputs) -> np.ndarray:
    from concourse.bass_utils import run_bass_kernel_spmd

    in_maps, bs = _prep_maps(inputs)
    nc = _get_program()
    res = run_bass_kernel_spmd(nc, in_maps, list(range(N_CORES))).results
    return _finish_output(res, bs)
